# revision 1
# baseline (speedup 1.0000x reference)
import sys

sys.path.insert(0, "/opt/trn_rl_repo")

import numpy as np
import ml_dtypes

import concourse.bacc as bacc
import concourse.bass as bass
import concourse.mybir as mybir
import concourse.tile as tile
from concourse.bass_utils import run_bass_kernel_spmd

F32 = mybir.dt.float32
BF16 = mybir.dt.bfloat16
AF = mybir.ActivationFunctionType
ALU = mybir.AluOpType
AX = mybir.AxisListType

# Problem constants (hardcoded per harness contract).
B, C, H, W = 4, 64, 128, 128
COUT1 = 128
NT = 9          # 3x3 taps
NFF = 4         # factor*factor subpixels
NCORES = 8
HL = H // 2     # 64 coarse rows per core
NYB = 4         # y-blocks for the weighted sum
YB = HL // NYB  # 16 rows per block

_cached = {}


def ap_of(t, off, dims):
    base = t[:]
    return bass.AP(base.tensor, base.offset + off, dims)


def build_nc():
    nc = bacc.Bacc("TRN2", target_bir_lowering=False, debug=False, num_devices=NCORES)

    hp2_d = nc.dram_tensor("hp2", [128, 66 * 130], F32, kind="ExternalInput")
    h8_d = nc.dram_tensor("h8", [64, 66 * 130], BF16, kind="ExternalInput")
    w1a_d = nc.dram_tensor("w1a", [128, 3 * 128], F32, kind="ExternalInput")
    w1b_d = nc.dram_tensor("w1b", [64, 3 * 128], F32, kind="ExternalInput")
    b1_d = nc.dram_tensor("b1c", [128, 1], F32, kind="ExternalInput")
    w2t_d = nc.dram_tensor("w2t", [128, 36], F32, kind="ExternalInput")
    b2_d = nc.dram_tensor("b2c", [36, 1], F32, kind="ExternalInput")
    sel_d = nc.dram_tensor("sel", [36, 4], F32, kind="ExternalInput")
    idf_d = nc.dram_tensor("idf", [128, 128], F32, kind="ExternalInput")
    idb_d = nc.dram_tensor("idb", [128, 128], BF16, kind="ExternalInput")
    out_d = nc.dram_tensor("out", [64, H, 2 * W], F32, kind="ExternalOutput")

    NA = 4 * YB          # acc free per c: (ff, y_local)
    n = YB * 64          # per-(tap,block) product elements per partition

    with tile.TileContext(nc) as tc:
        with (
            tc.tile_pool(name="const", bufs=1) as cpool,
            tc.tile_pool(name="ring", bufs=2) as ring,
            tc.tile_pool(name="mchunk", bufs=3) as mpool,
            tc.tile_pool(name="ws1", bufs=2) as wp2,
            tc.tile_pool(name="ws2", bufs=1) as wp1,
            tc.tile_pool(name="orow", bufs=3) as opool,
            tc.tile_pool(name="ps1", bufs=2, space=bass.MemorySpace.PSUM) as pp1,
            tc.tile_pool(name="ps2", bufs=2, space=bass.MemorySpace.PSUM) as pp2,
            tc.tile_pool(name="psz", bufs=1, space=bass.MemorySpace.PSUM) as ppz,
            tc.tile_pool(name="pst", bufs=1, space=bass.MemorySpace.PSUM) as ppt,
            tc.tile_pool(name="psh", bufs=1, space=bass.MemorySpace.PSUM) as pph,
            tc.tile_pool(name="pso", bufs=1, space=bass.MemorySpace.PSUM) as ppo,
        ):
            # ---- constants ----
            w1a = cpool.tile([128, 3 * 128], F32)
            w1b = cpool.tile([64, 3 * 128], F32)
            b1 = cpool.tile([128, 1], F32)
            w2t = cpool.tile([128, 36], F32)
            b2 = cpool.tile([36, 1], F32)
            sel = cpool.tile([36, 4], F32)
            idf = cpool.tile([128, 128], F32)
            idb = cpool.tile([128, 128], BF16)
            nc.sync.dma_start(w1a[:], w1a_d[:])
            nc.sync.dma_start(w1b[:], w1b_d[:])
            nc.sync.dma_start(b1[:], b1_d[:])
            nc.sync.dma_start(w2t[:], w2t_d[:])
            nc.sync.dma_start(b2[:], b2_d[:])
            nc.sync.dma_start(sel[:], sel_d[:])
            nc.sync.dma_start(idf[:], idf_d[:])
            nc.sync.dma_start(idb[:], idb_d[:])

            for yb in range(NYB):
                r0 = yb * YB  # first coarse row of this block
                hp2b = ring.tile([128, 18 * 130], F32, tag="hp2b")
                h8b = ring.tile([64, 18 * 130], BF16, tag="h8b")
                nc.sync.dma_start(hp2b[:], hp2_d[:, r0 * 130:(r0 + 18) * 130])
                nc.sync.dma_start(h8b[:], h8_d[:, r0 * 130:(r0 + 18) * 130])

                # ---- conv1 -> relu -> conv2 -> exp -> Z -> recip (4 chunks) ----
                eb = ring.tile([36, 4 * 512], F32, tag="eb")
                rzb = ring.tile([4, 4 * 512], F32, tag="rzb")
                for ic in range(4):
                    ps1 = pp1.tile([128, 512], F32)
                    for dy in range(3):
                        rhs = ap_of(hp2b, (4 * ic + dy) * 130,
                                    [[18 * 130, 128], [130, 4], [1, 128]])
                        nc.tensor.matmul(ps1[:], w1a[:, dy * 128:(dy + 1) * 128], rhs,
                                         start=(dy == 0), stop=False)
                    for dy in range(3):
                        rhs = ap_of(hp2b, (4 * ic + dy) * 130 + 2,
                                    [[18 * 130, 64], [130, 4], [1, 128]])
                        nc.tensor.matmul(ps1[:], w1b[:, dy * 128:(dy + 1) * 128], rhs,
                                         start=False, stop=(dy == 2))
                    m = mpool.tile([128, 512], F32)
                    nc.scalar.activation(m[:], ps1[:], AF.Relu, bias=b1[:], scale=1.0)
                    ps2 = pp2.tile([40, 512], F32)
                    nc.tensor.matmul(ps2[0:36, :], w2t[:], m[:])
                    nc.scalar.activation(eb[:, ic * 512:(ic + 1) * 512],
                                         ps2[0:36, :], AF.Exp, bias=b2[:], scale=1.0)
                    psz = ppz.tile([4, 512], F32)
                    nc.tensor.matmul(psz[:], sel[:], eb[0:36, ic * 512:(ic + 1) * 512])
                    nc.vector.reciprocal(rzb[:, ic * 512:(ic + 1) * 512], psz[:])

                # ---- h transposes (bf16), batched PSUM->SBUF copies ----
                hTb = ring.tile([128, 3 * 18 * 64], BF16, tag="hTb")
                for dx in range(3):
                    for j in range(3):   # 3 batches of 6 rows
                        psh = pph.tile([128, 6 * 64], BF16)
                        for r in range(6):
                            yp = j * 6 + r
                            nc.tensor.transpose(
                                psh[:, r * 64:(r + 1) * 64],
                                ap_of(h8b, yp * 130 + dx, [[18 * 130, 64], [1, 128]]),
                                idb[0:64, 0:64])
                        nc.scalar.copy(
                            hTb[:, (dx * 18 + j * 6) * 64:(dx * 18 + j * 6 + 6) * 64],
                            psh[:])

                # ---- e/rz transposes, batched ----
                eTb = ring.tile([128, YB * 40], F32, tag="eTb")
                for j in range(4):       # 4 batches of 4 rows
                    pst = ppt.tile([128, 4 * 40], F32)
                    for r in range(4):
                        yl = j * 4 + r
                        nc.tensor.transpose(pst[:, r * 40:r * 40 + 36],
                                            eb[:, yl * 128:(yl + 1) * 128],
                                            idf[0:36, 0:36])
                        nc.tensor.transpose(pst[:, r * 40 + 36:r * 40 + 40],
                                            rzb[:, yl * 128:(yl + 1) * 128],
                                            idf[0:4, 0:4])
                    nc.scalar.copy(eTb[:, j * 160:(j + 1) * 160], pst[:])

                # ---- normalized mask, transposed+duplicated (bf16) ----
                nmb = ring.tile([128, YB * 72], BF16, tag="nmb")
                for ff in range(NFF):
                    out_ap = ap_of(nmb, ff * 18, [[YB * 72, 128], [72, YB], [2, 9], [1, 2]])
                    in0 = ap_of(eTb, ff * 9, [[YB * 40, 128], [40, YB], [1, 9], [0, 2]])
                    in1 = ap_of(eTb, 36 + ff, [[YB * 40, 128], [40, YB], [0, 9], [0, 2]])
                    nc.vector.tensor_tensor(out_ap, in0, in1, ALU.mult)

                # ---- weighted tap sum (DVE, bf16) ----
                acc = ring.tile([128, 64 * NA], F32, tag="acc")  # (c, ff, yl)
                for ff in range(NFF):
                    prod = wp2.tile([128, NT * n], BF16, tag="prod")
                    for dy in range(3):
                        for dx in range(3):
                            t = dy * 3 + dx
                            in0 = ap_of(hTb, (dx * 18 + dy) * 64,
                                        [[3 * 18 * 64, 128], [64, YB], [2, 32], [1, 2]])
                            in1 = ap_of(nmb, (ff * 9 + t) * 2,
                                        [[YB * 72, 128], [72, YB], [0, 32], [1, 2]])
                            po = ap_of(prod, t * n,
                                       [[NT * n, 128], [64, YB], [2, 32], [1, 2]])
                            nc.vector.tensor_tensor(po, in0, in1, ALU.mult)
                    tA = wp2.tile([128, 4 * n], BF16, tag="tA")
                    tB = wp1.tile([128, 2 * n], BF16, tag="tB")
                    tC = wp1.tile([128, n], BF16, tag="tC")
                    nc.vector.tensor_add(tA[:], prod[:, 0:4 * n], prod[:, 4 * n:8 * n])
                    nc.vector.tensor_add(tB[:], tA[:, 0:2 * n], tA[:, 2 * n:4 * n])
                    nc.vector.tensor_add(tC[:], tB[:, 0:n], tB[:, n:2 * n])
                    acc_ap = ap_of(acc, ff * YB, [[64 * NA, 128], [1, YB], [NA, 64]])
                    tC_ap = ap_of(tC, 0, [[n, 128], [64, YB], [1, 64]])
                    p8_ap = ap_of(prod, 8 * n, [[NT * n, 128], [64, YB], [1, 64]])
                    nc.vector.tensor_tensor(acc_ap, tC_ap, p8_ap, ALU.add)

                # ---- pixel shuffle out ----
                for yl in range(YB):
                    y = yb * YB + yl
                    orow = opool.tile([128, 256], F32)
                    for fx in range(2):
                        pso = ppo.tile([128, 128], F32)
                        in_ap = ap_of(acc, fx * YB + yl,
                                      [[64 * NA, 128], [NA, 64], [2 * YB, 2]])
                        nc.tensor.transpose(pso[:], in_ap, idf[:])
                        o_ap = ap_of(orow, fx, [[256, 128], [2, 128]])
                        nc.scalar.copy(o_ap, pso[:])
                    nc.sync.dma_start(out_d[:, 2 * y:2 * y + 2, :], orow[:])

    nc.compile()
    return nc


def prep_shared(W1, b1, W2, b2):
    W1 = np.asarray(W1, np.float32)
    b1 = np.asarray(b1, np.float32)
    W2 = np.asarray(W2, np.float32).reshape(36, 128)
    b2 = np.asarray(b2, np.float32)

    w1a = np.zeros((128, 3 * 128), np.float32)
    w1b = np.zeros((64, 3 * 128), np.float32)
    for dy in range(3):
        w1a[0:64, dy * 128:(dy + 1) * 128] = W1[:, :, dy, 0].T
        w1a[64:128, dy * 128:(dy + 1) * 128] = W1[:, :, dy, 1].T
        w1b[:, dy * 128:(dy + 1) * 128] = W1[:, :, dy, 2].T

    o_of_mp = np.array([t * 4 + ff for ff in range(4) for t in range(9)])
    w2t = np.ascontiguousarray((0.25 * W2[o_of_mp, :]).T)
    b2c = np.ascontiguousarray((0.25 * b2[o_of_mp]).reshape(36, 1))

    sel = np.zeros((36, 4), np.float32)
    for k in range(36):
        sel[k, k // 9] = 1.0
    idf = np.eye(128, dtype=np.float32)
    return {
        "w1a": w1a, "w1b": w1b, "b1c": b1.reshape(128, 1).astype(np.float32),
        "w2t": w2t.astype(np.float32), "b2c": b2c, "sel": sel, "idf": idf,
        "idb": np.eye(128, dtype=ml_dtypes.bfloat16),
    }


def kernel(h, W1, b1, W2, b2, _trace=False):
    h = np.asarray(h, np.float32)
    shared = prep_shared(W1, b1, W2, b2)

    hp = np.pad(h, ((0, 0), (0, 0), (1, 1), (1, 1)))  # [B, C, 130, 130]
    in_maps = []
    for core in range(NCORES):
        b, half = core // 2, core % 2
        y0 = half * HL
        win = hp[b, :, y0:y0 + 66, :]  # [64, 66, 130]
        hp2 = np.zeros((128, 66, 130), np.float32)
        hp2[0:64] = win
        hp2[64:128, :, 0:129] = win[:, :, 1:130]
        h8 = (8.0 * win).astype(np.float32)
        m = dict(shared)
        m["hp2"] = hp2.reshape(128, -1)
        m["h8"] = np.ascontiguousarray(h8.reshape(64, -1)).astype(ml_dtypes.bfloat16)
        in_maps.append(m)

    if "nc" not in _cached:
        _cached["nc"] = build_nc()
    res = run_bass_kernel_spmd(_cached["nc"], in_maps, core_ids=list(range(NCORES)),
                               trace=_trace)

    out = np.zeros((B, C, 2 * H, 2 * W), np.float32)
    for core in range(NCORES):
        b, half = core // 2, core % 2
        out[b, :, half * 128:(half + 1) * 128, :] = res.results[core]["out"]
    if _trace:
        return out, res
    return out



# revision 7
# speedup vs baseline: 1.9893x; 1.9893x over previous
import sys

sys.path.insert(0, "/opt/trn_rl_repo")

import numpy as np
import ml_dtypes

import concourse.bacc as bacc
import concourse.bass as bass
import concourse.mybir as mybir
import concourse.tile as tile
from concourse.bass_utils import run_bass_kernel_spmd

F32 = mybir.dt.float32
BF16 = mybir.dt.bfloat16
AF = mybir.ActivationFunctionType
ALU = mybir.AluOpType
AX = mybir.AxisListType

# Problem constants (hardcoded per harness contract).
B, C, H, W = 4, 64, 128, 128
COUT1 = 128
NT = 9          # 3x3 taps
NFF = 4         # factor*factor subpixels
NCORES = 8
HL = H // 2     # 64 coarse rows per core
NYB = 4         # y-blocks
YB = HL // NYB  # 16 rows per block
N1 = YB * 64    # per-(tap,ff) product elements per partition (16 rows x 64 c)
SPLIT_K = 6     # taps 0..SPLIT_K-1 summed on PE via identity matmuls

_cached = {}


def ap_of(t, off, dims):
    base = t[:]
    return bass.AP(base.tensor, base.offset + off, dims)


def build_nc():
    nc = bacc.Bacc("TRN2", target_bir_lowering=False, debug=False, num_devices=NCORES)

    hp2_d = nc.dram_tensor("hp2", [128, 66 * 130], BF16, kind="ExternalInput")
    ht3_d = nc.dram_tensor("ht3", [128, 66 * 192], BF16, kind="ExternalInput")
    w1a_d = nc.dram_tensor("w1a", [128, 3 * 128], BF16, kind="ExternalInput")
    w1b_d = nc.dram_tensor("w1b", [64, 3 * 128], BF16, kind="ExternalInput")
    b1_d = nc.dram_tensor("b1c", [128, 1], F32, kind="ExternalInput")
    w2t_d = nc.dram_tensor("w2t", [128, 36], BF16, kind="ExternalInput")
    eb2_d = nc.dram_tensor("eb2d", [128, 72], BF16, kind="ExternalInput")
    idq_d = nc.dram_tensor("idq", [128, 128], BF16, kind="ExternalInput")
    out_d = nc.dram_tensor("out", [64, H * 2 * W], F32, kind="ExternalOutput")

    with tile.TileContext(nc) as tc:
        with (
            tc.tile_pool(name="const", bufs=1) as cpool,
            tc.tile_pool(name="ring", bufs=2) as ring,
            tc.tile_pool(name="mpool", bufs=2) as mpool,
            tc.tile_pool(name="spool", bufs=2) as spool,
            tc.tile_pool(name="prodp", bufs=2) as prodp,
            tc.tile_pool(name="dpool", bufs=2) as dpool,
            tc.tile_pool(name="accp", bufs=2) as accp,
            tc.tile_pool(name="orow", bufs=3) as opool,
            tc.tile_pool(name="ps1", bufs=2, space=bass.MemorySpace.PSUM) as pp1,
            tc.tile_pool(name="psE", bufs=2, space=bass.MemorySpace.PSUM) as ppE,
            tc.tile_pool(name="psA", bufs=3, space=bass.MemorySpace.PSUM) as ppA,
            tc.tile_pool(name="pso", bufs=1, space=bass.MemorySpace.PSUM) as ppo,
        ):
            # ---- constants ----
            w1a = cpool.tile([128, 3 * 128], BF16)
            w1b = cpool.tile([64, 3 * 128], BF16)
            b1 = cpool.tile([128, 1], F32)
            w2t = cpool.tile([128, 36], BF16)
            eb2 = cpool.tile([128, 72], BF16)
            idq = cpool.tile([128, 128], BF16)
            nc.sync.dma_start(w1a[:], w1a_d[:])
            nc.sync.dma_start(w1b[:], w1b_d[:])
            nc.sync.dma_start(b1[:], b1_d[:])
            nc.sync.dma_start(w2t[:], w2t_d[:])
            nc.sync.dma_start(eb2[:], eb2_d[:])
            nc.sync.dma_start(idq[:], idq_d[:])

            for yb in range(NYB):
                r0 = yb * YB  # first coarse row of this block
                hp2b = ring.tile([128, 18 * 130], BF16, tag="hp2b")
                ht3b = ring.tile([128, 18 * 192], BF16, tag="ht3b")
                nc.sync.dma_start(hp2b[:], hp2_d[:, r0 * 130:(r0 + 18) * 130])
                nc.sync.dma_start(ht3b[:], ht3_d[:, r0 * 192:(r0 + 18) * 192])

                # ---- conv1 -> relu -> conv2(rows) -> exp(dup) ----
                m = mpool.tile([128, 2048], BF16, tag="m")
                eT2 = spool.tile([128, YB * 72], BF16, tag="eT2")
                for ic in range(4):
                    ps1 = pp1.tile([128, 512], F32)
                    for dy in range(3):
                        rhs = ap_of(hp2b, (4 * ic + dy) * 130,
                                    [[18 * 130, 128], [130, 4], [1, 128]])
                        nc.tensor.matmul(ps1[:], w1a[:, dy * 128:(dy + 1) * 128], rhs,
                                         start=(dy == 0), stop=False)
                    for dy in range(3):
                        rhs = ap_of(hp2b, (4 * ic + dy) * 130 + 2,
                                    [[18 * 130, 64], [130, 4], [1, 128]])
                        nc.tensor.matmul(ps1[:], w1b[:, dy * 128:(dy + 1) * 128], rhs,
                                         start=False, stop=(dy == 2))
                    nc.scalar.activation(m[:, ic * 512:(ic + 1) * 512], ps1[:],
                                         AF.Relu, bias=b1[:], scale=1.0)
                    psE = ppE.tile([128, 160], F32)
                    for rl in range(4):
                        r = 4 * ic + rl
                        nc.tensor.matmul(psE[:, rl * 40:rl * 40 + 36],
                                         m[:, r * 128:(r + 1) * 128], w2t[:])
                    # exp with free-dup x2: eT2[x, (4r, 36, 2)]
                    e_out = ap_of(eT2, ic * 4 * 72,
                                  [[YB * 72, 128], [72, 4], [2, 36], [1, 2]])
                    e_in = ap_of(psE, 0, [[160, 128], [40, 4], [1, 36], [0, 2]])
                    nc.scalar.activation(e_out, e_in, AF.Exp, scale=1.0)

                # ---- softmax pieces (transposed layout, x on partitions) ----
                q2 = spool.tile([128, YB * 72], BF16, tag="q2")
                in_e = ap_of(eT2, 0, [[YB * 72, 128], [1, YB * 72]])
                in_b = ap_of(eb2, 0, [[72, 128], [0, YB], [1, 72]])
                q_out = ap_of(q2, 0, [[YB * 72, 128], [72, YB], [1, 72]])
                nc.vector.tensor_tensor(q_out, in_e, in_b, ALU.mult)

                zt = spool.tile([128, 64], F32, tag="zt")
                rz = spool.tile([128, 64], F32, tag="rz")
                rzd = spool.tile([128, 128], BF16, tag="rzd")
                z_in = ap_of(q2, 0, [[YB * 72, 128], [72, YB], [18, 4], [2, 9]])
                nc.vector.tensor_reduce(zt[:], z_in, AX.X, ALU.add)
                nc.vector.reciprocal(rz[:], zt[:])
                rzd_out = ap_of(rzd, 0, [[128, 128], [2, 64], [1, 2]])
                rzd_in = ap_of(rz, 0, [[64, 128], [1, 64], [0, 2]])
                nc.scalar.copy(rzd_out, rzd_in)

                # nm[x, (ff, r, t, 2)] = q2 * rz  (bf16, dup x2 for 2x mode)
                nm = spool.tile([128, NFF * YB * 18], BF16, tag="nm")
                for ff in range(NFF):
                    o = ap_of(nm, ff * YB * 18, [[NFF * YB * 18, 128], [18, YB], [1, 18]])
                    i0 = ap_of(q2, ff * 18, [[YB * 72, 128], [72, YB], [1, 18]])
                    i1 = ap_of(rzd, ff * 2, [[128, 128], [8, YB], [0, 9], [1, 2]])
                    nc.vector.tensor_tensor(o, i0, i1, ALU.mult)

                # ---- tap products + split tap-sum ----
                acc = accp.tile([128, NFF * N1], BF16, tag="acc")
                for ff in range(NFF):
                    prod = prodp.tile([128, NT * N1], BF16, tag="prod")
                    for t in range(NT):
                        dy, dx = t // 3, t % 3
                        i0 = ap_of(ht3b, dy * 192 + dx * 64,
                                   [[18 * 192, 128], [192, YB], [2, 32], [1, 2]])
                        i1 = ap_of(nm, ff * YB * 18 + t * 2,
                                   [[NFF * YB * 18, 128], [18, YB], [0, 32], [1, 2]])
                        po = ap_of(prod, t * N1,
                                   [[NT * N1, 128], [64, YB], [2, 32], [1, 2]])
                        nc.vector.tensor_tensor(po, i0, i1, ALU.mult)
                    # DVE tree over taps SPLIT_K..8  ->  D [128, N1]
                    nrem = NT - SPLIT_K
                    tD = dpool.tile([128, 2 * N1], BF16, tag="tD")
                    if nrem == 3:
                        nc.vector.tensor_add(tD[:, 0:N1],
                                             prod[:, 6 * N1:7 * N1],
                                             prod[:, 7 * N1:8 * N1])
                        nc.vector.tensor_add(tD[:, N1:2 * N1],
                                             tD[:, 0:N1], prod[:, 8 * N1:9 * N1])
                        dfin = tD[:, N1:2 * N1]
                    else:
                        raise NotImplementedError
                    # PE: psacc = sum(prod[0..SPLIT_K-1]) + D, two 512-halves
                    for half in range(2):
                        psacc = ppA.tile([128, 512], F32)
                        for t in range(SPLIT_K):
                            rhs = ap_of(prod, t * N1 + half * 512,
                                        [[NT * N1, 128], [1, 512]])
                            nc.tensor.matmul(psacc[:], idq[:], rhs,
                                             start=(t == 0), stop=False)
                        rhs = ap_of(tD, N1 + half * 512, [[2 * N1, 128], [1, 512]])
                        nc.tensor.matmul(psacc[:], idq[:], rhs,
                                         start=False, stop=True)
                        # acc layout: [x, (r, fx, fy, c)]; ff = fy*2 + fx
                        fy, fx = ff // 2, ff % 2
                        a_out = ap_of(acc, (half * 8) * 256 + fx * 128 + fy * 64,
                                      [[NFF * N1, 128], [256, 8], [1, 64]])
                        a_in = ap_of(psacc, 0, [[512, 128], [64, 8], [1, 64]])
                        nc.scalar.copy(a_out, a_in)

                # ---- pixel shuffle out ----
                for yg in range(4):
                    psoB = ppo.tile([128, 1024], BF16)
                    for yl_loc in range(4):
                        yl = yg * 4 + yl_loc
                        for fx in range(2):
                            t_in = ap_of(acc, yl * 256 + fx * 128,
                                         [[NFF * N1, 128], [1, 128]])
                            nc.tensor.transpose(
                                psoB[:, yl_loc * 256 + fx * 128:
                                     yl_loc * 256 + (fx + 1) * 128],
                                t_in, idq[:])
                    orow4 = opool.tile([128, 1024], F32)
                    for fx in range(2):
                        o_ap = ap_of(orow4, fx, [[1024, 128], [256, 4], [2, 128]])
                        i_ap = ap_of(psoB, fx * 128, [[1024, 128], [256, 4], [1, 128]])
                        nc.scalar.copy(o_ap, i_ap)
                    for fy in range(2):
                        od = ap_of(out_d, (2 * r0 + 8 * yg + fy) * 256,
                                   [[2 * H * W, 64], [512, 4], [1, 256]])
                        nc.sync.dma_start(od, orow4[fy * 64:(fy + 1) * 64, :])

    nc.compile()
    return nc


def prep_shared(W1, b1, W2, b2):
    W1 = np.asarray(W1, np.float32)
    b1 = np.asarray(b1, np.float32)
    W2 = np.asarray(W2, np.float32).reshape(36, 128)
    b2 = np.asarray(b2, np.float32)

    w1a = np.zeros((128, 3 * 128), np.float32)
    w1b = np.zeros((64, 3 * 128), np.float32)
    for dy in range(3):
        w1a[0:64, dy * 128:(dy + 1) * 128] = W1[:, :, dy, 0].T
        w1a[64:128, dy * 128:(dy + 1) * 128] = W1[:, :, dy, 1].T
        w1b[:, dy * 128:(dy + 1) * 128] = W1[:, :, dy, 2].T

    # w2t columns k = ff*9 + t  ->  original channel t*4 + ff, 0.25 folded in
    o_of_mp = np.array([t * 4 + ff for ff in range(4) for t in range(9)])
    w2t = np.ascontiguousarray((0.25 * W2[o_of_mp, :]).T)
    eb2 = np.exp(0.25 * b2[o_of_mp]).astype(np.float32)        # [36]
    eb2d = np.broadcast_to(np.repeat(eb2, 2)[None, :], (128, 72))

    bf = ml_dtypes.bfloat16
    return {
        "w1a": w1a.astype(bf), "w1b": w1b.astype(bf),
        "b1c": b1.reshape(128, 1).astype(np.float32),
        "w2t": w2t.astype(bf),
        "eb2d": np.ascontiguousarray(eb2d).astype(bf),
        "idq": np.eye(128, dtype=bf),
    }


def kernel(h, W1, b1, W2, b2, _trace=False):
    h = np.asarray(h, np.float32)
    shared = prep_shared(W1, b1, W2, b2)

    hp = np.pad(h, ((0, 0), (0, 0), (1, 1), (1, 1)))  # [B, C, 130, 130]
    bf = ml_dtypes.bfloat16
    in_maps = []
    for core in range(NCORES):
        b, half = core // 2, core % 2
        y0 = half * HL
        win = hp[b, :, y0:y0 + 66, :]  # [64, 66, 130]
        hp2 = np.zeros((128, 66, 130), np.float32)
        hp2[0:64] = win
        hp2[64:128, :, 0:129] = win[:, :, 1:130]
        # ht3[x, y, dx, c] = 8*win[c, y, x+dx]
        w8 = 8.0 * win
        ht3 = np.stack([w8[:, :, dx:dx + 128] for dx in range(3)],
                       axis=0).transpose(3, 2, 0, 1)  # [128, 66, 3, 64]
        m = dict(shared)
        m["hp2"] = np.ascontiguousarray(hp2.reshape(128, -1)).astype(bf)
        m["ht3"] = np.ascontiguousarray(ht3.reshape(128, -1)).astype(bf)
        in_maps.append(m)

    if "nc" not in _cached:
        _cached["nc"] = build_nc()
    res = run_bass_kernel_spmd(_cached["nc"], in_maps, core_ids=list(range(NCORES)),
                               trace=_trace)

    out = np.zeros((B, C, 2 * H, 2 * W), np.float32)
    for core in range(NCORES):
        b, half = core // 2, core % 2
        out[b, :, half * 128:(half + 1) * 128, :] = \
            res.results[core]["out"].reshape(64, H, 2 * W)
    if _trace:
        return out, res
    return out
